# revision 1
# baseline (speedup 1.0000x reference)
"""Trainium2 Bass kernel for LocalBackwardTemporalAttention.

Sharding: data-parallel over batch B=8 across the 8 NeuronCores (one batch
element per core, no collectives). Per-core pipeline (feature-major "fm" =
[features-on-partitions, tokens-free]; token-major "tm" = transpose):

  A: LayerNorm(x) -> kv_ln (fm bf16 + tm bf16), q_ln (fm bf16)
  B: QKV projections -> q_fm, k_fm (fm), v_tm (tm)   [1/sqrt(D) folded into wq]
  C: per (t,head) attention: S=q@k^T -> softmax -> PE-transpose -> attn^T@v -> o_fm
  D: out_proj -> att_fm
  E: mlpq GEMM1(+exact gelu) -> h1q; GEMM2 in swapped (token-major-out)
     orientation + bias + kv_ln residual -> qpre_tm (f32)
  F: res_ln + ln2 (token-major) + PE-transpose -> ln2t_fm
  G: mlp GEMM1(+gelu) -> h1; GEMM2 swapped orientation + bias -> out (f32,
     token-major, written directly in final layout)

All GEMMs run with bf16 inputs / f32 PSUM accumulation (validated ~3.4e-3
absmax-relative error vs the f32 reference). LN/softmax statistics in f32.
"""

import sys

sys.path.insert(0, "/opt/trn_rl_repo")

import numpy as np
import ml_dtypes

import concourse.bass as bass
import concourse.bacc as bacc_mod
import concourse.mybir as mybir
import concourse.tile as tile
from concourse.masks import make_identity

F32 = mybir.dt.float32
BF16 = mybir.dt.bfloat16
AF = mybir.ActivationFunctionType
ALU = mybir.AluOpType
AX = mybir.AxisListType

# problem shapes (hardcoded per spec)
B, HW, NF, E, M, H = 8, 16, 196, 1024, 4096, 16
T, D = HW - 1, E // H            # 15, 64
NKV, NQ = T * NF, NF             # 2940, 196
NTOK = HW * NF                   # 3136
EPS = 1e-6
P = 128
NT = 490                         # token n-tile for fm GEMMs (2940 = 6*490)
KE, KM = E // P, M // P          # 8, 32 k-chunks
ME, MM = E // P, M // P          # m-tiles


def _ceil(a, b):
    return -(-a // b)


def _bcast_ap(handle, n):
    """[n-partition broadcast of a 1-D dram tensor] -> AP [n, len]"""
    a = handle[:]
    return bass.AP(tensor=a.tensor, offset=a.offset, ap=[[0, n], list(a.ap[0])])


def _col_ap(handle, mo):
    """1-D dram tensor (mo*128,) viewed as [128, mo] with elem (p, m) = m*128+p"""
    a = handle[:]
    return bass.AP(tensor=a.tensor, offset=a.offset, ap=[[1, P], [P, mo]])


def build_nc():
    nc = bacc_mod.Bacc(None, target_bir_lowering=False)
    t = lambda n, s, d: nc.dram_tensor(n, s, d, kind="ExternalInput")

    x = t("x", [NTOK, E], F32)
    wqT = t("wqT", [E, E], BF16)
    wkT = t("wkT", [E, E], BF16)
    wvT = t("wvT", [E, E], BF16)
    woT = t("woT", [E, E], BF16)
    w1qT = t("w1qT", [E, M], BF16)
    w2qT = t("w2qT", [M, E], BF16)
    w1T = t("w1T", [E, M], BF16)
    w2T = t("w2T", [M, E], BF16)
    bqs = t("bqs", [E], F32)
    bk = t("bk", [E], F32)
    bv = t("bv", [E], F32)
    bo = t("bo", [E], F32)
    b1q = t("b1q", [M], F32)
    b2q = t("b2q", [E], F32)
    b1 = t("b1", [M], F32)
    b2 = t("b2", [E], F32)
    gq = t("gq", [E], F32)
    bbq = t("bbq", [E], F32)
    gkv = t("gkv", [E], F32)
    bbkv = t("bbkv", [E], F32)
    gres = t("gres", [E], F32)
    bbres = t("bbres", [E], F32)
    gln2 = t("gln2", [E], F32)
    bbln2 = t("bbln2", [E], F32)
    out = nc.dram_tensor("out", [NKV, E], F32, kind="ExternalOutput")

    with tile.TileContext(nc) as tc:
        with tc.tile_pool(name="dram", bufs=1, space="DRAM") as dram, \
             tc.tile_pool(name="consts", bufs=1) as consts:
            kv_fm = dram.tile([E, NKV], BF16)
            kv_tm = dram.tile([NKV, E], BF16)
            q_fm = dram.tile([E, NQ], BF16)
            k_fm = dram.tile([E, NKV], BF16)
            v_tm = dram.tile([NKV, E], BF16)
            o_fm = dram.tile([E, NKV], BF16)
            att_fm = dram.tile([E, NKV], BF16)
            h1q_fm = dram.tile([M, NKV], BF16)
            qpre_tm = dram.tile([NKV, E], F32)
            ln2t_fm = dram.tile([E, NKV], BF16)
            h1_fm = dram.tile([M, NKV], BF16)

            ident = consts.tile([P, P], BF16)
            make_identity(nc, ident)
            epst = consts.tile([P, 1], F32)
            nc.vector.memset(epst, EPS)

            # ---------------- stage A: LN + transpose ----------------
            def ln_pass(xin_rows, gain, bias_, n_rows, fm_out, tm_out):
                with tc.tile_pool(name="ln", bufs=3) as pool, \
                     tc.tile_pool(name="lnst", bufs=4) as stp, \
                     tc.tile_pool(name="lnps", bufs=4, space="PSUM") as psp, \
                     tc.tile_pool(name="lng", bufs=1) as gp:
                    gt = gp.tile([P, E], F32, name="ln_gain")
                    bt = gp.tile([P, E], F32, name="ln_bias")
                    nc.sync.dma_start(out=gt, in_=_bcast_ap(gain, P))
                    nc.sync.dma_start(out=bt, in_=_bcast_ap(bias_, P))
                    for it in range(_ceil(n_rows, P)):
                        r0 = it * P
                        p = min(P, n_rows - r0)
                        xt = pool.tile([P, E], F32, name="ln_x")
                        nc.sync.dma_start(
                            out=xt[:p], in_=xin_rows[r0:r0 + p, :])
                        x3 = xt.rearrange("p (n f) -> p n f", n=2)
                        st = stp.tile([P, 2, 6], F32, name="ln_st")
                        for i in range(2):
                            nc.vector.bn_stats(out=st[:p, i, :], in_=x3[:p, i, :])
                        mv = stp.tile([P, 2], F32, name="ln_mv")
                        nc.vector.bn_aggr(out=mv[:p], in_=st[:p])
                        rs = stp.tile([P, 1], F32, name="ln_rs")
                        nc.scalar.activation(out=rs[:p], in_=mv[:p, 1:2],
                                             func=AF.Sqrt, bias=epst[:p])
                        nc.vector.reciprocal(out=rs[:p], in_=rs[:p])
                        y = pool.tile([P, E], BF16, name="ln_y")
                        nc.vector.tensor_scalar(
                            out=y[:p], in0=xt[:p], scalar1=mv[:p, 0:1],
                            scalar2=rs[:p], op0=ALU.subtract, op1=ALU.mult)
                        nc.vector.tensor_mul(y[:p], y[:p], gt[:p])
                        nc.vector.tensor_add(y[:p], y[:p], bt[:p])
                        if tm_out is not None:
                            nc.sync.dma_start(out=tm_out[r0:r0 + p, :], in_=y[:p])
                        tp = psp.tile([P, KE, P], BF16, name="ln_tp")
                        for e in range(KE):
                            nc.tensor.transpose(
                                out=tp[:, e, :p],
                                in_=y[:p, e * P:(e + 1) * P],
                                identity=ident[:p, :p])
                        fmt = pool.tile([P, KE, P], BF16, name="ln_fmt")
                        nc.scalar.copy(out=fmt, in_=tp)
                        dst = fm_out[:, r0:r0 + p].rearrange(
                            "(e r) c -> r e c", r=P)
                        nc.sync.dma_start(out=dst, in_=fmt[:, :, :p])

            ln_pass(x[:NKV, :], gkv, bbkv, NKV, kv_fm, kv_tm)
            ln_pass(x[NKV:, :], gq, bbq, NQ, q_fm, None)

            # ---------------- fm GEMM helper ----------------
            # out_fm[mo*P, n] = act(wT[K, mo*P].T @ x_fm[K, n] + bias_col)
            def gemm_fm(wT, x_fm_ap, n_total, kc, mo, bias_h, out_fm, act, tagp):
                with tc.tile_pool(name=tagp + "w", bufs=1) as wp, \
                     tc.tile_pool(name=tagp + "x", bufs=3) as xp, \
                     tc.tile_pool(name=tagp + "o", bufs=4) as op, \
                     tc.tile_pool(name=tagp + "ps", bufs=4, space="PSUM") as pp, \
                     tc.tile_pool(name=tagp + "b", bufs=1) as bp:
                    wsb = wp.tile([P, kc, mo * P], BF16, name=tagp + "_w")
                    for k in range(kc):
                        nc.sync.dma_start(
                            out=wsb[:, k, :],
                            in_=wT[k * P:(k + 1) * P, :])
                    bsb = bp.tile([P, mo], F32, name=tagp + "_b")
                    nc.sync.dma_start(out=bsb, in_=_col_ap(bias_h, mo))
                    nts = _ceil(n_total, NT)
                    for n in range(nts):
                        n0 = n * NT
                        w = min(NT, n_total - n0)
                        xt = xp.tile([P, kc, NT], BF16, name=tagp + "_x")
                        nc.sync.dma_start(
                            out=xt[:, :, :w],
                            in_=x_fm_ap[:, n0:n0 + w].rearrange(
                                "(k r) c -> r k c", r=P))
                        for m in range(mo):
                            ps = pp.tile([P, NT], F32, name=tagp + "_ps")
                            for k in range(kc):
                                nc.tensor.matmul(
                                    ps[:, :w],
                                    lhsT=wsb[:, k, m * P:(m + 1) * P],
                                    rhs=xt[:, k, :w],
                                    start=(k == 0), stop=(k == kc - 1))
                            ev = op.tile([P, NT], BF16, name=tagp + "_ev")
                            nc.scalar.activation(
                                out=ev[:, :w], in_=ps[:, :w], func=act,
                                bias=bsb[:, m:m + 1])
                            nc.sync.dma_start(
                                out=out_fm[m * P:(m + 1) * P, n0:n0 + w],
                                in_=ev[:, :w])

            # ---------------- stage B: QKV ----------------
            gemm_fm(wkT[:, :], kv_fm, NKV, KE, ME, bk, k_fm, AF.Identity, "kp")
            gemm_fm(wqT[:, :], q_fm, NQ, KE, ME, bqs, q_fm, AF.Identity, "qp")

            # v (token-major out): v_tm[c0:c1, :] = kv_fm[:, c0:c1].T @ wvT
            def gemm_tm(lhs_fm, kc, rhsT, n_out, bias_free_h, out_tm, resid_tm,
                        out_dt, tagp):
                nb = n_out // 512
                with tc.tile_pool(name=tagp + "w", bufs=1) as wp, \
                     tc.tile_pool(name=tagp + "h", bufs=3) as hp, \
                     tc.tile_pool(name=tagp + "o", bufs=4) as op, \
                     tc.tile_pool(name=tagp + "r", bufs=3) as rp, \
                     tc.tile_pool(name=tagp + "ps", bufs=4, space="PSUM") as pp, \
                     tc.tile_pool(name=tagp + "b", bufs=1) as bp:
                    wsb = wp.tile([P, kc, n_out], BF16, name=tagp + "_w")
                    for k in range(kc):
                        nc.sync.dma_start(
                            out=wsb[:, k, :],
                            in_=rhsT[k * P:(k + 1) * P, :])
                    bsb = bp.tile([P, n_out], F32, name=tagp + "_b")
                    nc.sync.dma_start(out=bsb, in_=_bcast_ap(bias_free_h, P))
                    for c in range(_ceil(NKV, P)):
                        c0 = c * P
                        p = min(P, NKV - c0)
                        ht = hp.tile([P, kc, P], BF16, name=tagp + "_h")
                        nc.sync.dma_start(
                            out=ht[:, :, :p],
                            in_=lhs_fm[:, c0:c0 + p].rearrange(
                                "(k r) c -> r k c", r=P))
                        rt = None
                        if resid_tm is not None:
                            rt = rp.tile([P, n_out], BF16, name=tagp + "_r")
                            nc.sync.dma_start(out=rt[:p],
                                              in_=resid_tm[c0:c0 + p, :])
                        for j in range(nb):
                            ps = pp.tile([P, 512], F32, name=tagp + "_ps")
                            for k in range(kc):
                                nc.tensor.matmul(
                                    ps[:p, :],
                                    lhsT=ht[:, k, :p],
                                    rhs=wsb[:, k, j * 512:(j + 1) * 512],
                                    start=(k == 0), stop=(k == kc - 1))
                            ev = op.tile([P, 512], out_dt, name=tagp + "_ev")
                            nc.vector.tensor_add(
                                ev[:p], ps[:p, :], bsb[:p, j * 512:(j + 1) * 512])
                            if rt is not None:
                                nc.vector.tensor_add(
                                    ev[:p], ev[:p], rt[:p, j * 512:(j + 1) * 512])
                            nc.sync.dma_start(
                                out=out_tm[c0:c0 + p, j * 512:(j + 1) * 512],
                                in_=ev[:p])

            gemm_tm(kv_fm, KE, wvT[:, :], E, bv, v_tm, None, BF16, "vp")

            # ---------------- stage C: attention ----------------
            with tc.tile_pool(name="cq", bufs=1) as cqp, \
                 tc.tile_pool(name="ckv", bufs=4) as ckv, \
                 tc.tile_pool(name="cat", bufs=3) as cat, \
                 tc.tile_pool(name="cst", bufs=4) as cst, \
                 tc.tile_pool(name="co", bufs=3) as cop, \
                 tc.tile_pool(name="cps", bufs=2, space="PSUM") as cps, \
                 tc.tile_pool(name="cpt", bufs=2, space="PSUM") as cpt, \
                 tc.tile_pool(name="cpo", bufs=2, space="PSUM") as cpo:
                qsb = cqp.tile([P, KE, NQ], BF16)
                nc.sync.dma_start(
                    out=qsb, in_=q_fm[:, :].rearrange("(e r) c -> r e c", r=P))
                nch = [(0, P), (P, NQ - P)]           # n/m chunks: 128 + 68
                for t_ in range(T):
                    t0 = t_ * NF
                    osb = cop.tile([P, KE, NQ], BF16, name="c_osb")
                    for hp in range(H // 2):
                      ksb = ckv.tile([P, NQ], BF16, name="c_k")
                      nc.sync.dma_start(
                          out=ksb,
                          in_=k_fm[hp * P:(hp + 1) * P, t0:t0 + NF])
                      vsb = ckv.tile([P, 2, P], BF16, name="c_v")
                      for j, (m0, mj) in enumerate(nch):
                          nc.sync.dma_start(
                              out=vsb[:mj, j, :],
                              in_=v_tm[t0 + m0:t0 + m0 + mj,
                                       hp * P:(hp + 1) * P])
                      for pi in range(2):
                        d0 = pi * D
                        # S = q @ k^T  (scale already folded into wq)
                        ps = cps.tile([P, 2, 512], F32, name="c_ps")
                        for j, (n0, pn) in enumerate(nch):
                            nc.tensor.matmul(
                                ps[:pn, j, :NQ],
                                lhsT=qsb[d0:d0 + D, hp, n0:n0 + pn],
                                rhs=ksb[d0:d0 + D, :],
                                start=True, stop=True)
                        nm = cst.tile([P, 2], F32, name="c_nm")
                        nc.vector.reduce_max(
                            out=nm, in_=ps[:, :, :NQ], axis=AX.X, negate=True)
                        asb = cat.tile([P, 2, NQ], BF16, name="c_asb")
                        sm = cst.tile([P, 2], F32, name="c_sm")
                        for j, (n0, pn) in enumerate(nch):
                            nc.scalar.activation(
                                out=asb[:pn, j, :], in_=ps[:pn, j, :NQ],
                                func=AF.Exp, bias=nm[:pn, j:j + 1],
                                accum_out=sm[:pn, j:j + 1])
                        rc = cst.tile([P, 2], F32, name="c_rc")
                        nc.vector.reciprocal(out=rc, in_=sm)
                        for j, (n0, pn) in enumerate(nch):
                            nc.vector.tensor_scalar_mul(
                                asb[:pn, j, :], in0=asb[:pn, j, :],
                                scalar1=rc[:pn, j:j + 1])
                        # transpose attn -> attnT
                        pt = cpt.tile([P, 2, NQ], BF16, name="c_pt")
                        for jn, (n0, pn) in enumerate(nch):
                            for jm, (m0, mj) in enumerate(nch):
                                nc.tensor.transpose(
                                    out=pt[:mj, jm, n0:n0 + pn],
                                    in_=asb[:pn, jn, m0:m0 + mj],
                                    identity=ident[:pn, :pn])
                        atT = cat.tile([P, 2, NQ], BF16, name="c_atT")
                        nc.scalar.copy(out=atT, in_=pt)
                        # o^T[d, n] = sum_m v[m, d] * attnT[m, n]
                        po = cpo.tile([D, NQ], F32, name="c_po")
                        for jm, (m0, mj) in enumerate(nch):
                            nc.tensor.matmul(
                                po[:, :],
                                lhsT=vsb[:mj, jm, d0:d0 + D],
                                rhs=atT[:mj, jm, :],
                                start=(jm == 0), stop=(jm == 1))
                        nc.scalar.copy(out=osb[d0:d0 + D, hp, :],
                                       in_=po)
                    nc.sync.dma_start(
                        out=o_fm[:, t0:t0 + NF].rearrange(
                            "(e r) c -> r e c", r=P),
                        in_=osb)

            # ---------------- stage D: out_proj ----------------
            gemm_fm(woT[:, :], o_fm, NKV, KE, ME, bo, att_fm, AF.Identity, "op")

            # ---------------- stage E: mlpq ----------------
            gemm_fm(w1qT[:, :], att_fm, NKV, KE, MM, b1q, h1q_fm, AF.Gelu, "e1")
            gemm_tm(h1q_fm, KM, w2qT[:, :], E, b2q, qpre_tm, kv_tm, F32, "e2")

            # ---------------- stage F: res_ln + ln2 + transpose ----------------
            with tc.tile_pool(name="f", bufs=3) as fp, \
                 tc.tile_pool(name="fst", bufs=4) as fst, \
                 tc.tile_pool(name="fps", bufs=4, space="PSUM") as fps, \
                 tc.tile_pool(name="fg", bufs=1) as fg:
                g1 = fg.tile([P, E], F32)
                bb1 = fg.tile([P, E], F32)
                g2 = fg.tile([P, E], F32)
                bb2 = fg.tile([P, E], F32)
                nc.sync.dma_start(out=g1, in_=_bcast_ap(gres, P))
                nc.sync.dma_start(out=bb1, in_=_bcast_ap(bbres, P))
                nc.sync.dma_start(out=g2, in_=_bcast_ap(gln2, P))
                nc.sync.dma_start(out=bb2, in_=_bcast_ap(bbln2, P))
                for it in range(_ceil(NKV, P)):
                    r0 = it * P
                    p = min(P, NKV - r0)
                    xt = fp.tile([P, E], F32, name="f_x")
                    nc.sync.dma_start(out=xt[:p], in_=qpre_tm[r0:r0 + p, :])
                    cur = xt
                    for li, (gg, bb) in enumerate(((g1, bb1), (g2, bb2))):
                        x3 = cur.rearrange("p (n f) -> p n f", n=2)
                        st = fst.tile([P, 2, 6], F32, name="f_st")
                        for i in range(2):
                            nc.vector.bn_stats(out=st[:p, i, :], in_=x3[:p, i, :])
                        mv = fst.tile([P, 2], F32, name="f_mv")
                        nc.vector.bn_aggr(out=mv[:p], in_=st[:p])
                        rs = fst.tile([P, 1], F32, name="f_rs")
                        nc.scalar.activation(out=rs[:p], in_=mv[:p, 1:2],
                                             func=AF.Sqrt, bias=epst[:p])
                        nc.vector.reciprocal(out=rs[:p], in_=rs[:p])
                        y = fp.tile([P, E], F32 if li == 0 else BF16,
                                    name=f"f_y{li}")
                        nc.vector.tensor_scalar(
                            out=y[:p], in0=cur[:p], scalar1=mv[:p, 0:1],
                            scalar2=rs[:p], op0=ALU.subtract, op1=ALU.mult)
                        nc.vector.tensor_mul(y[:p], y[:p], gg[:p])
                        nc.vector.tensor_add(y[:p], y[:p], bb[:p])
                        cur = y
                    tp = fps.tile([P, KE, P], BF16, name="f_tp")
                    for e in range(KE):
                        nc.tensor.transpose(
                            out=tp[:, e, :p],
                            in_=cur[:p, e * P:(e + 1) * P],
                            identity=ident[:p, :p])
                    fmt = fp.tile([P, KE, P], BF16, name="f_fmt")
                    nc.scalar.copy(out=fmt, in_=tp)
                    nc.sync.dma_start(
                        out=ln2t_fm[:, r0:r0 + p].rearrange(
                            "(e r) c -> r e c", r=P),
                        in_=fmt[:, :, :p])

            # ---------------- stage G: mlp ----------------
            gemm_fm(w1T[:, :], ln2t_fm, NKV, KE, MM, b1, h1_fm, AF.Gelu, "g1")
            gemm_tm(h1_fm, KM, w2T[:, :], E, b2, out[:, :], None, F32, "g2")

    nc.compile()
    return nc


_NC = None


def _get_nc():
    global _NC
    if _NC is None:
        _NC = build_nc()
    return _NC


def _prep_in_maps(inputs):
    f32 = lambda a: np.ascontiguousarray(np.asarray(a, dtype=np.float32))
    bf = lambda a: np.ascontiguousarray(
        np.asarray(a, dtype=np.float32).astype(ml_dtypes.bfloat16))
    x = f32(inputs["inputs"])                       # (B,HW,NF,E)
    ipw = f32(inputs["in_proj_w"])
    ipb = f32(inputs["in_proj_b"])
    wq, wk, wv = ipw[:E], ipw[E:2 * E], ipw[2 * E:]
    bq, bk_, bv_ = ipb[:E], ipb[E:2 * E], ipb[2 * E:]
    s = 1.0 / np.sqrt(np.float32(D))
    shared = {
        "wqT": bf(wq.T * s), "wkT": bf(wk.T), "wvT": bf(wv.T),
        "woT": bf(f32(inputs["out_proj_w"]).T),
        "w1qT": bf(f32(inputs["mlpq_w1"]).T),
        "w2qT": bf(f32(inputs["mlpq_w2"]).T),
        "w1T": bf(f32(inputs["mlp_w1"]).T),
        "w2T": bf(f32(inputs["mlp_w2"]).T),
        "bqs": f32(bq * s), "bk": f32(bk_), "bv": f32(bv_),
        "bo": f32(inputs["out_proj_b"]),
        "b1q": f32(inputs["mlpq_b1"]), "b2q": f32(inputs["mlpq_b2"]),
        "b1": f32(inputs["mlp_b1"]), "b2": f32(inputs["mlp_b2"]),
        "gq": f32(inputs["ln_q_g"]), "bbq": f32(inputs["ln_q_b"]),
        "gkv": f32(inputs["ln_kv_g"]), "bbkv": f32(inputs["ln_kv_b"]),
        "gres": f32(inputs["res_ln_g"]), "bbres": f32(inputs["res_ln_b"]),
        "gln2": f32(inputs["ln2_g"]), "bbln2": f32(inputs["ln2_b"]),
    }
    return [dict(shared, x=np.ascontiguousarray(x[b].reshape(NTOK, E)))
            for b in range(B)]


def _run(inputs, trace=False):
    from concourse.bass_utils import run_bass_kernel_spmd
    nc = _get_nc()
    in_maps = _prep_in_maps(inputs)
    res = run_bass_kernel_spmd(nc, in_maps, core_ids=list(range(B)),
                               trace=trace)
    outs = np.stack([r["out"].reshape(T, NF, E) for r in res.results])
    return outs, res


def kernel(**inputs) -> np.ndarray:
    outs, _ = _run(inputs, trace=False)
    return outs

